# revision 32
# baseline (speedup 1.0000x reference)
"""Trainium2 Bass kernel for the 3-layer LSTM + MLP head (nn_CustomLSTM).

Strategy (pure data parallelism, batch 512 -> 64 per core x 8 cores):

Device layout: everything transposed — [features on partitions, batch on
free dim]. The three LSTM layers run as a wavefront over time (at tick tau,
layer1 computes t=tau, layer2 t=tau-1, layer3 t=tau-2) so the per-step
elementwise work of all three layers packs into single wide instructions
([*, 192] instead of 3x [*, 64]), which is what beats the per-instruction
overhead wall of the 768-step sequential recurrence.

Per tick, one PSUM bank [128, 384] holds all six gate blocks
(cols [L1ig|L2ig|L3ig|L1fo|L2fo|L3fo], rows [i;g] / [f;o]), filled by:
  - one identity matmul adding the (constant) biases,
  - two identity matmuls adding layer1's precomputed input projection xg1,
  - per-layer recurrence matmuls; layers 2/3 use K=128 "cat" weights
    [Whh_l | Wih_l] against rhs [h_l[t-1] ; h_{l-1}[t]].
The g-rows carry 2x-scaled weights so one sigmoid over the whole bank +
(2s-1) gives tanh for g (sigmoid's partition-relocating variant handles the
base-partition constraints).

xg1 = Wih1 @ x^T is precomputed in 512-column chunks on the tensor engine,
streamed ~8 ticks ahead of consumption.
"""
import numpy as np
import ml_dtypes
from contextlib import ExitStack

BF = ml_dtypes.bfloat16

import concourse.bass as bass
import concourse.tile as tile
from concourse import mybir
from concourse.bass_utils import run_bass_kernel_spmd

F32 = mybir.dt.float32
BF16 = mybir.dt.bfloat16
AF = mybir.ActivationFunctionType
ALU = mybir.AluOpType

B_FULL, T_FULL, D_IN, H, N_CLS = 512, 256, 258, 64, 90
N_CORES = 8
BL = B_FULL // N_CORES  # 64 batch per core

_FC_DIMS = [(64, H), (128, 64), (64, 128), (32, 64), (32, 32), (N_CLS, 32)]


def _split_multiwaits(nc, limit=1):
    """The walrus codegen only supports one semaphore wait per instruction;
    Tile's final drain can carry several. Split extras onto NoOps.

    The wait kept on the real instruction is chosen to be the FRESHEST one
    (smallest satisfied-ago distance): stale WAR waits go to leading NoOps
    that clear instantly, so the instruction dispatches as soon as its real
    dependency fires instead of serializing behind a NoOp's dispatch."""
    for fn in nc.m.functions:
        for bb in fn.blocks:
            emitted = {}
            out = []
            for inst in bb.instructions:
                si = inst.sync_info
                if si is not None and si.on_wait and len(si.on_wait) > limit:
                    waits = list(si.on_wait)

                    def stale(w):
                        if w.wait_value is None:
                            return 0
                        return emitted.get(w.id, 0) - w.wait_value

                    waits.sort(key=stale)          # freshest first
                    spill = waits[limit:]
                    spill.sort(key=stale, reverse=True)   # stalest earliest
                    for i in range(0, len(spill), limit):
                        nop = mybir.InstNoOp(
                            name=nc.get_next_instruction_name(),
                            engine=inst.engine, ins=[], outs=[])
                        nop.sync_info = mybir.SyncInfo(
                            on_wait=spill[i:i + limit], on_update=[])
                        nc.register_instruction(nop)
                        out.append(nop)
                    si.on_wait = waits[:limit]
                out.append(inst)
                if si is not None:
                    for u in si.on_update:
                        emitted[u.id] = emitted.get(u.id, 0) + 1
            bb.instructions[:] = out


def _build(T=T_FULL, loop_n=None, ablate=0):
    assert T % 8 == 0
    NCHUNK = T // 8          # phase-A chunks of 512 cols (8 ticks each)
    NCOLS = T * BL
    nc = bass.Bass()

    xT_d = nc.dram_tensor("xT", [D_IN, NCOLS], BF16, kind="ExternalInput")
    # packed weights: minimize DMA count (each DMA pays ~10.4us of SWDGE
    # issue latency; the prologue queue gates the first ticks)
    wApk_d = nc.dram_tensor("wApk", [128, 512], BF16, kind="ExternalInput")
    wAk2_d = nc.dram_tensor("wAk2", [2, 256], BF16, kind="ExternalInput")
    bigw_d = nc.dram_tensor("bigw", [128, 1792], BF16, kind="ExternalInput")
    fcwpk_d = nc.dram_tensor("fcwpk", [128, 410], BF16, kind="ExternalInput")
    fcbpk_d = nc.dram_tensor("fcbpk", [128, 6], F32, kind="ExternalInput")
    out_d = nc.dram_tensor("out", [N_CLS, BL], F32, kind="ExternalOutput")

    with tile.TileContext(nc) as tc, ExitStack() as ctx:
        import contextlib
        const = ctx.enter_context(tc.tile_pool(name="const", bufs=1))
        xgpool = ctx.enter_context(tc.tile_pool(name="xg", bufs=NCHUNK))
        xstage = ctx.enter_context(tc.tile_pool(name="xstage", bufs=5))
        psA = ctx.enter_context(tc.tile_pool(name="psA", bufs=2, space="PSUM"))
        bank = ctx.enter_context(tc.tile_pool(name="bank", bufs=2, space="PSUM"))
        work = ctx.enter_context(tc.tile_pool(name="work", bufs=8))
        cats = ctx.enter_context(tc.tile_pool(name="cats", bufs=8))

        dma = nc.sync.dma_start

        # ---- constants -------------------------------------------------
        # wA first: phase-A (the first PE work) depends only on it + x
        wApk = const.tile([128, 512], BF16, tag="wApk")
        dma(out=wApk, in_=wApk_d[:])
        wAk2 = const.tile([2, 256], BF16, tag="wAk2")
        dma(out=wAk2, in_=wAk2_d[:])
        wA_ig = [wApk[:, 0:128], wApk[:, 128:256], wAk2[:, 0:128]]
        wA_fo = [wApk[:, 256:384], wApk[:, 384:512], wAk2[:, 128:256]]

        bigw = const.tile([128, 1792], BF16, tag="bigw")
        ident = bigw[:, 0:128]
        pbias = bigw[:, 128:512]
        w1 = bigw[:, 512:768]                             # data at p64-127
        rw = {n: bigw[:, 768 + i * 256:1024 + i * 256]    # data at p64-127
              for i, n in enumerate(("wih2", "whh2", "wih3", "whh3"))}

        D = const.tile([128, 192], BF16, tag="D")         # c/2 state at p64-127
        nc.vector.memset(D, 0.0)

        loop_cm = tc.For_i(0, loop_n, 1) if loop_n else contextlib.nullcontext()
        Rconst = const.tile([128, 192], BF16, tag="Rconst")
        nc.vector.memset(Rconst, 0.0)

        # ---- phase A: xg1 chunks --------------------------------------
        xg_ig = [None] * NCHUNK
        xg_fo = [None] * NCHUNK

        _psA_t = {}

        def phase_a_dma(j, stage):
            xa = xstage.tile([128, 512], BF16, tag="xa")
            dma(out=xa, in_=xT_d[0:128, j * 512:(j + 1) * 512])
            xb = xstage.tile([128, 512], BF16, tag="xb")
            dma(out=xb, in_=xT_d[128:256, j * 512:(j + 1) * 512])
            xc = xstage.tile([2, 512], BF16, tag="xc")
            dma(out=xc, in_=xT_d[256:258, j * 512:(j + 1) * 512])
            stage[j] = (xa, xb, xc)

        def phase_a_step(j, stage, step):
            # One quarter of a chunk: one gate-block, one 256-col half.
            # Emitted at the END of a tick so the PE/DVE work drains in the
            # idle window while the next tick's chain is still blocked.
            blk, half = step // 2, step % 2
            wset = wA_ig if blk == 0 else wA_fo
            dst = xg_ig if blk == 0 else xg_fo
            xa, xb, xc = stage[j]
            c0, c1 = half * 256, (half + 1) * 256
            if half == 0:
                _psA_t[blk] = psA.tile([128, 512], F32, tag=f"psA{blk}",
                                       name=f"psA{blk}t")
            p = _psA_t[blk]
            nc.tensor.matmul(p[:, c0:c1], lhsT=wset[0], rhs=xa[:, c0:c1],
                             start=True, stop=False, skip_group_check=True)
            nc.tensor.matmul(p[:, c0:c1], lhsT=wset[1], rhs=xb[:, c0:c1],
                             start=False, stop=False, skip_group_check=True)
            nc.tensor.matmul(p[:, c0:c1], lhsT=wset[2], rhs=xc[:, c0:c1],
                             start=False, stop=True, skip_group_check=True)
            if half == 0:
                g = xgpool.tile([128, 512], BF16, tag=f"xg{blk}",
                                name=f"xg{blk}_{j}")
                dst[j] = g
            _pend_cast[(j, step)] = (dst[j][:, c0:c1], p[:, c0:c1])

        _pend_cast = {}

        def phase_a_cast(j, step):
            # the PSUM->SBUF copy is emitted ticks after its matmuls so it
            # never sits unsatisfied ahead of chain ops in the DVE FIFO
            g_ap, p_ap = _pend_cast.pop((j, step))
            nc.vector.tensor_copy(out=g_ap, in_=p_ap)

        with loop_cm:
            xstage_t = {}
            phase_a_dma(0, xstage_t)
            dma(out=bigw, in_=bigw_d[:])
            for jj in range(1, min(4, NCHUNK)):
                phase_a_dma(jj, xstage_t)
            for s in range(4):
                phase_a_step(0, xstage_t, s)
                phase_a_cast(0, s)
            if NCHUNK > 1:
                for s in range(4):
                    phase_a_step(1, xstage_t, s)
                    phase_a_cast(1, s)

            # ---- wavefront over ticks -------------------------------------
            # Bank is split into two PSUM tiles so sigmoid(X) can fire while
            # the Y-block recurrence matmuls are still streaming:
            #   PX [128,192] cols [L1|L2|L3], rows [i; f]
            #   PY [128,192] cols [L1|L2|L3], rows [2g; o]
            # Cell state kept as D = c/2 so the update is a plain TT add
            # (bf16 2x mode) and tanh(c) = Tanh(D, scale=2) for free.
            def offchain_mms(tau, PX, PY):
                # Bias + xg matmuls for tick tau's banks: independent of the
                # recurrence chain, emitted a tick early so the PE runs them
                # while stalled waiting for R23.
                nc.tensor.matmul(PX, lhsT=ident, rhs=pbias[:, 0:192],
                                 start=True, stop=False, skip_group_check=True)
                nc.tensor.matmul(PY, lhsT=ident, rhs=pbias[:, 192:384],
                                 start=True, stop=False, skip_group_check=True)
                if tau <= T - 1:
                    ch, off = tau // 8, (tau % 8) * 64
                    nc.tensor.matmul(PX[:, 0:64], lhsT=ident,
                                     rhs=xg_ig[ch][:, off:off + 64],
                                     start=False, stop=False, skip_group_check=True)
                    nc.tensor.matmul(PY[:, 0:64], lhsT=ident,
                                     rhs=xg_fo[ch][:, off:off + 64],
                                     start=False, stop=False, skip_group_check=True)

            # H tile per tick [128,192], data only at p64-127:
            #   cols 0:64 = h1[tau], 64:128 = h2[tau-1], 128:192 = h3[tau-2].
            # All recurrence matmuls are K=64 against slices of H, so no
            # partition-relocating TT is needed: TT1 writes [h1|h2], a small
            # TT writes h3, all at p64-127.
            Hc = None
            H3 = None
            Pnext = None
            for tau in range(T + 2):
                l1 = tau <= T - 1
                l2 = 1 <= tau <= T
                l3 = 2 <= tau <= T + 1

                if Pnext is None:
                    PX = bank.tile([128, 192], F32, tag="PX")
                    PY = bank.tile([128, 192], F32, tag="PY")
                    offchain_mms(tau, PX, PY)
                else:
                    PX, PY = Pnext
                # X block first (h1/h2-dependent MMs first: they only need
                # TT1), so sigmoid(X) can issue while the Y matmuls stream.
                for P, c0 in ((PX, 0), (PY, 128)):
                    if l1 and tau >= 1:
                        nc.tensor.matmul(P[:, 0:64], lhsT=w1[64:128, c0:c0 + 128],
                                         rhs=Hc[64:128, 0:64], start=False,
                                         stop=False, skip_group_check=True)
                    if l2:
                        nc.tensor.matmul(P[:, 64:128],
                                         lhsT=rw["wih2"][64:128, c0:c0 + 128],
                                         rhs=Hc[64:128, 0:64], start=False,
                                         stop=False, skip_group_check=True)
                        nc.tensor.matmul(P[:, 64:128],
                                         lhsT=rw["whh2"][64:128, c0:c0 + 128],
                                         rhs=Hc[64:128, 64:128], start=False,
                                         stop=False, skip_group_check=True)
                    if l3:
                        nc.tensor.matmul(P[:, 128:192],
                                         lhsT=rw["wih3"][64:128, c0:c0 + 128],
                                         rhs=Hc[64:128, 64:128], start=False,
                                         stop=False, skip_group_check=True)
                        nc.tensor.matmul(P[:, 128:192],
                                         lhsT=rw["whh3"][64:128, c0:c0 + 128],
                                         rhs=Hc[64:128, 128:192], start=False,
                                         stop=True, skip_group_check=True)
                if tau + 1 < T + 2:
                    Pnext = (bank.tile([128, 192], F32, tag="PX", name="PXn"),
                             bank.tile([128, 192], F32, tag="PY", name="PYn"))
                    offchain_mms(tau + 1, *Pnext)
                else:
                    Pnext = None

                # elementwise, all three layers packed [*, 192]
                #   i = GAX[0:64, :]   f = GAX[64:128, :]
                #   s = GAY[0:64, :]   o = GAY[64:128, :]
                GAX = work.tile([128, 192], BF16, tag="GAX")
                GAY = work.tile([128, 192], BF16, tag="GAY")
                if ablate < 3:
                    nc.scalar.activation(GAX, PX, AF.Sigmoid)
                    nc.scalar.activation(GAY, PY, AF.Sigmoid)
                if ablate >= 2:
                    Hc = Rconst
                    continue
                # V = f * d  (needs only sigmoid(X): overlaps sigmoid(Y))
                V = work.tile([128, 192], BF16, tag="V")
                nc.vector.tensor_tensor(out=V[64:128, :], in0=GAX[64:128, :],
                                        in1=D[64:128, :], op=ALU.mult)
                # U = i*(s-0.5) = i*tanh(ghat)/2, relocated to p64-127
                U = work.tile([128, 192], BF16, tag="U")
                nc.vector.scalar_tensor_tensor(
                    out=U[64:128, :], in0=GAY[0:64, :], scalar=-0.5,
                    in1=GAX[0:64, :], op0=ALU.add, op1=ALU.mult)
                # d' = f*d + i*(s-0.5)  (= c/2)
                nc.vector.tensor_tensor(out=D[64:128, :], in0=U[64:128, :],
                                        in1=V[64:128, :], op=ALU.add)
                TC = work.tile([128, 192], BF16, tag="TC")    # data at p64-127
                nc.scalar.activation(TC[64:128, :], D[64:128, :],
                                     AF.Tanh, scale=2.0)

                # h = o * tanh(c): next tick's rhs tile [h1|h2|h3] at p64-127
                Hn = cats.tile([128, 192], BF16, tag="Hn")
                nc.vector.tensor_tensor(out=Hn[64:128, :],
                                        in0=GAY[64:128, :],
                                        in1=TC[64:128, :], op=ALU.mult)

                # cell-state resets + zero-h overrides at layer-start ticks
                if tau == 0:
                    nc.vector.memset(D[64:128, 64:128], 0.0)
                    nc.gpsimd.memset(Hn[64:128, 64:128], 0.0)    # h2[-1] = 0
                elif tau == 1:
                    nc.vector.memset(D[64:128, 128:192], 0.0)
                    nc.gpsimd.memset(Hn[64:128, 128:192], 0.0)   # h3[-1] = 0
                Hc = Hn if ablate == 0 else Rconst
                if tau == T + 1:
                    H3 = Hn  # h3[T-1] = Hn[64:128, 128:192]

                # one quarter of the next xg chunk per tick, emitted last so
                # its PE/DVE work fills idle windows instead of blocking the
                # recurrence stream
                if tau % 8 == 4 and tau // 8 + 4 < NCHUNK:
                    phase_a_dma(tau // 8 + 4, xstage_t)
                if tau % 2 == 0:
                    j = tau // 8 + 2
                    if j < NCHUNK:
                        phase_a_step(j, xstage_t, (tau % 8) // 2)
                else:
                    r = tau % 8
                    if r == 1:
                        jc, sc = tau // 8 + 1, 3
                    else:
                        jc, sc = tau // 8 + 2, (r - 3) // 2
                    if 2 <= jc < NCHUNK:
                        phase_a_cast(jc, sc)
                if tau == max(0, T - 16):
                    # FC-head weights, fetched mid-loop so the DMA latency
                    # is hidden long before the epilogue needs them
                    fcwpk = const.tile([128, 410], BF16, tag="fcwpk")
                    dma(out=fcwpk, in_=fcwpk_d[:])
                    fcbpk = const.tile([128, 6], F32, tag="fcbpk")
                    dma(out=fcbpk, in_=fcbpk_d[:])

            # ---- FC head ---------------------------------------------------
            fcw_s = []
            fcb_s = []
            c0 = 0
            for i, (m, k) in enumerate(_FC_DIMS):
                if i == 0:  # rhs is h3 at base partition 64
                    fcw_s.append(fcwpk[64:128, c0:c0 + m])
                else:
                    fcw_s.append(fcwpk[0:k, c0:c0 + m])
                fcb_s.append(fcbpk[0:m, i:i + 1])
                c0 += m

            if ablate >= 2:
                H3 = Rconst
            z = H3[64:128, 128:192]   # h3[T-1], base partition 64
            for i, (m, k) in enumerate(_FC_DIMS):
                pz = bank.tile([m, 64], F32, tag="PX", name="fcp")
                nc.tensor.matmul(pz, lhsT=fcw_s[i], rhs=z, start=True, stop=True)
                zs = work.tile([m, 64], F32 if i == 5 else BF16, tag=f"fz{i}")
                func = AF.Relu if i < 5 else AF.Identity
                nc.scalar.activation(zs, pz, func, bias=fcb_s[i])
                z = zs
            dma(out=out_d[:], in_=z)

    _split_multiwaits(nc)
    return nc


_BUILT = {}


def _get_nc(T=T_FULL, loop_n=None, ablate=0):
    key = (T, loop_n, ablate)
    if key not in _BUILT:
        _BUILT[key] = _build(T, loop_n, ablate)
    return _BUILT[key]


def _sel_ig(W):
    # block X: rows [i; f]
    return np.concatenate([W[0:H], W[H:2 * H]], axis=0)


def _sel_fo(W):
    # block Y: rows [2g; o]
    return np.concatenate([2.0 * W[2 * H:3 * H], W[3 * H:4 * H]], axis=0)


def _prep_weights(inp, T):
    """Host-side weight/bias rearrangement shared by all cores."""
    f32 = np.float32
    Wih1, Whh1 = inp["Wih1"].astype(f32), inp["Whh1"].astype(f32)
    wA = np.concatenate(
        [_sel_ig(Wih1).T, _sel_fo(Wih1).T], axis=1)            # [258, 256]
    w1 = np.concatenate(
        [_sel_ig(Whh1).T, _sel_fo(Whh1).T], axis=1)            # [64, 256]

    def xy(W):
        return np.concatenate([_sel_ig(W).T, _sel_fo(W).T], axis=1)  # [64,256]

    pbias = np.zeros((128, 384), f32)
    for l in range(3):
        b = (inp[f"bih{l+1}"] + inp[f"bhh{l+1}"]).astype(f32)
        big = np.concatenate([b[0:H], b[H:2 * H]])
        bfo = np.concatenate([2.0 * b[2 * H:3 * H], b[3 * H:4 * H]])
        pbias[:, l * 64:(l + 1) * 64] = big[:, None]
        pbias[:, 192 + l * 64:192 + (l + 1) * 64] = bfo[:, None]

    # packed dram images (fewest possible DMAs)
    wApk = np.zeros((128, 512), f32)
    wApk[:, 0:128] = wA[0:128, 0:128]
    wApk[:, 128:256] = wA[128:256, 0:128]
    wApk[:, 256:384] = wA[0:128, 128:256]
    wApk[:, 384:512] = wA[128:256, 128:256]
    wAk2 = np.concatenate([wA[256:258, 0:128], wA[256:258, 128:256]], axis=1)

    bigw = np.zeros((128, 1792), f32)
    bigw[:, 0:128] = np.eye(128, dtype=f32)
    bigw[:, 128:512] = pbias
    bigw[64:128, 512:768] = w1
    for i, n in enumerate(("Wih2", "Whh2", "Wih3", "Whh3")):
        bigw[64:128, 768 + i * 256:1024 + i * 256] = xy(inp[n].astype(f32))

    fcwpk = np.zeros((128, 410), f32)
    fcws = [inp[f"Wfc{i}"].astype(f32).T for i in range(1, 6)]
    fcws.append(inp["Wout"].astype(f32).T)
    fcbs = [inp[f"bfc{i}"].astype(f32) for i in range(1, 6)]
    fcbs.append(inp["bout"].astype(f32))
    fcbpk = np.zeros((128, 6), f32)
    c0 = 0
    for i, (m_, k_) in enumerate(_FC_DIMS):
        r0 = 64 if i == 0 else 0
        fcwpk[r0:r0 + k_, c0:c0 + m_] = fcws[i]
        fcbpk[0:m_, i] = fcbs[i]
        c0 += m_

    return {
        "wApk": np.ascontiguousarray(wApk.astype(BF)),
        "wAk2": np.ascontiguousarray(wAk2.astype(BF)),
        "bigw": np.ascontiguousarray(bigw.astype(BF)),
        "fcwpk": np.ascontiguousarray(fcwpk.astype(BF)),
        "fcbpk": np.ascontiguousarray(fcbpk),
    }


def run(inputs, trace=False, **rk):
    x = np.asarray(inputs["x"], np.float32)
    B, T, D = x.shape
    nc = _get_nc(T)
    shared = _prep_weights(inputs, T)

    bl = B // N_CORES
    in_maps = []
    for c in range(N_CORES):
        xc = x[c * bl:(c + 1) * bl]                    # [bl, T, D]
        xT = np.ascontiguousarray(
            xc.transpose(2, 1, 0).reshape(D, T * bl).astype(BF))
        in_maps.append({"xT": xT, **shared})

    bkr = run_bass_kernel_spmd(nc, in_maps, list(range(N_CORES)),
                               trace=trace, **rk)
    res = bkr.results
    out = np.empty((B, N_CLS), np.float32)
    for c in range(N_CORES):
        out[c * bl:(c + 1) * bl] = res[c]["out"].T
    return out, bkr


def kernel(**inputs):
    return run(inputs)[0]

